# revision 20
# baseline (speedup 1.0000x reference)
import sys
import os

sys.path.insert(0, "/opt/trn_rl_repo")

import numpy as np
import ml_dtypes

import concourse.bass as bass
import concourse.tile as tile
from concourse import mybir, library_config
from concourse.tile import add_dep_helper
from concourse.bass_utils import run_bass_kernel_spmd

# Problem constants (nn_MoEBlock: B,C,T,H,W = 2,128,8,64,64; E=8; top-2)
B, C, T, H, W = 2, 128, 8, 64, 64
E = 8
NVOX = B * T * H * W          # 65536 voxels
NCORES = 8
NSH = NVOX // NCORES          # 8192 voxels per core
NC_CHUNK = 1024               # main-loop chunk (voxels)
NCHUNKS = NSH // NC_CHUNK
F32 = mybir.dt.float32
F32R = mybir.dt.float32r
NEG_BIG = -1e30


def _r(ap):
    return ap.bitcast(F32R)



def _split_waits(nc, max_waits=1):
    """This walrus accepts only one sync-wait command per instruction.
    Move extra on_wait conditions onto standalone same-engine NoOps
    inserted immediately before the instruction (same engine stream =>
    identical semantics)."""
    ctr = 0
    for f in nc.m.functions:
        for bb in f.blocks:
            insts = list(bb.instructions)
            out = []
            changed = False
            for inst in insts:
                si = inst.sync_info
                w = list(si.on_wait) if si is not None and si.on_wait else []
                if (len(w) > max_waits
                        and inst.engine != mybir.EngineType.Unassigned):
                    for extra in w[:-max_waits]:
                        ctr += 1
                        nop = mybir.InstNoOp(
                            name=f"WSPLIT-{ctr}", ins=[], outs=[])
                        nop.engine = inst.engine
                        nop.sync_info = mybir.SyncInfo(
                            on_wait=[extra], on_update=[])
                        out.append(nop)
                    inst.sync_info = mybir.SyncInfo(
                        on_wait=w[-max_waits:],
                        on_update=list(si.on_update) if si.on_update else [])
                    changed = True
                out.append(inst)
            if changed:
                try:
                    bb.instructions = out
                except Exception:
                    bb.instructions.clear()
                    bb.instructions.extend(out)
    return nc


def build_kernel(hasgb: bool, hasb1: bool, hasb2: bool, act_fn=None):
    if act_fn is None:
        act_fn = mybir.ActivationFunctionType.Silu
    nc = bass.Bass()
    x_d = nc.dram_tensor("x", [C, NSH], F32R, kind="ExternalInput")
    gwT_d = nc.dram_tensor("gwT", [C, E], F32R, kind="ExternalInput")
    gb_d = nc.dram_tensor("gb", [C, E], F32, kind="ExternalInput")
    w1T_d = nc.dram_tensor("w1T", [C, E * C], F32R, kind="ExternalInput")
    b1_d = nc.dram_tensor("b1m", [C, E], F32, kind="ExternalInput")
    w2T_d = nc.dram_tensor("w2T", [C, E * C], mybir.dt.bfloat16, kind="ExternalInput")
    b2_d = nc.dram_tensor("b2m", [E, C], mybir.dt.bfloat16, kind="ExternalInput")
    id_d = nc.dram_tensor("ident", [C, C], F32, kind="ExternalInput")
    sel_d = nc.dram_tensor("sel", [E, E * C], mybir.dt.bfloat16, kind="ExternalInput")
    out_d = nc.dram_tensor("out", [C, NSH], F32, kind="ExternalOutput")

    with tile.TileContext(nc) as tc:
        with (
            tc.tile_pool(name="consts", bufs=1) as consts,
            tc.tile_pool(name="xp", bufs=1) as xp,
            tc.tile_pool(name="gat", bufs=1) as gat,
            tc.tile_pool(name="fpool", bufs=3) as fpool,
            tc.tile_pool(name="gpool", bufs=3) as gpool,
            tc.tile_pool(name="opool", bufs=2) as opool,
        ):
            # ---------- phase 0: loads ----------
            x_sb = xp.tile([C, NSH], F32R)
            gwT = consts.tile([C, E], F32R)
            gbr = consts.tile([C, E], F32)
            w1T = consts.tile([C, E * C], F32R)
            b1m = consts.tile([C, E], F32)
            w2T = consts.tile([C, E * C], mybir.dt.bfloat16)
            b2m = consts.tile([E, C], mybir.dt.bfloat16)
            ident = consts.tile([C, C], F32)
            scal1 = consts.tile([C, 1], F32)
            sel = consts.tile([E, E * C], mybir.dt.bfloat16)

            dmas = []
            for j in range(4):
                s = slice(j * (NSH // 4), (j + 1) * (NSH // 4))
                dmas.append(nc.sync.dma_start(x_sb[:, s], x_d[:, s]))
            dmas.append(nc.sync.dma_start(gwT[:], gwT_d[:]))
            dmas.append(nc.sync.dma_start(gbr[:], gb_d[:]))
            dmas.append(nc.sync.dma_start(w1T[:], w1T_d[:]))
            dmas.append(nc.sync.dma_start(b1m[:], b1_d[:]))
            dmas.append(nc.sync.dma_start(w2T[:], w2T_d[:]))
            dmas.append(nc.sync.dma_start(b2m[:], b2_d[:]))
            dmas.append(nc.sync.dma_start(ident[:], id_d[:]))
            nc.vector.memset(scal1[:], 1.0)
            dmas.append(nc.sync.dma_start(sel[:], sel_d[:]))

            # PE can carry only ONE sync wait per Matmult through walrus;
            # absorb each input-DMA dependency into a PE nop up front.
            dma_nops = []
            for dma in dmas:
                nop = nc.tensor.nop(nofuse=True)
                add_dep_helper(nop.ins, dma.ins, sync=True)
                dma_nops.append(nop)

            def pe_absorb(producers, consumer_mms):
                nops = []
                for p in producers:
                    if p is None:
                        continue
                    n = nc.tensor.nop(nofuse=True)
                    add_dep_helper(n.ins, p.ins, sync=True)
                    nops.append(n)
                for m in consumer_mms:
                    for n in nops:
                        add_dep_helper(m.ins, n.ins, sync=False)

            # ---------- phase G: gating ----------
            gpsum = tc.tile_pool(name="ps_g", bufs=1, space="PSUM")
            ps_l = gpsum.__enter__()
            NT = NSH // C  # 64 voxel tiles of 128
            psl = ps_l.tile([C, NT * E], F32)   # [128, 512] logits, voxel-major
            for i in range(NT):
                lmm = nc.tensor.matmul(
                    psl[:, i * E:(i + 1) * E],
                    x_sb[:, i * C:(i + 1) * C].bitcast(F32),
                    gwT[:].bitcast(F32),
                    start=True, stop=True,
                )
                if i == 0:
                    for n in dma_nops:
                        add_dep_helper(lmm.ins, n.ins, sync=False)
            l3 = psl[:].rearrange("p (t e) -> p t e", e=E)
            if hasgb:
                lsb = gat.tile([C, NT * E], F32)
                nc.vector.tensor_add(
                    lsb[:].rearrange("p (t e) -> p t e", e=E), l3,
                    gbr[:, None, :].broadcast_to((C, NT, E)))
                l3 = lsb[:].rearrange("p (t e) -> p t e", e=E)

            m1 = gat.tile([C, NT], F32)
            nc.vector.tensor_reduce(
                out=m1[:], in_=l3, op=mybir.AluOpType.max, axis=mybir.AxisListType.X)
            ge1 = gat.tile([C, NT * E], F32)
            g13 = ge1[:].rearrange("p (t e) -> p t e", e=E)
            nc.vector.tensor_tensor(
                g13, l3, m1[:, :, None].broadcast_to((C, NT, E)),
                op=mybir.AluOpType.is_ge)
            tneg = gat.tile([C, NT * E], F32)
            nc.vector.tensor_scalar_mul(tneg[:], ge1[:], NEG_BIG)
            lm = gat.tile([C, NT * E], F32)
            nc.vector.tensor_add(
                lm[:].rearrange("p (t e) -> p t e", e=E), l3,
                tneg[:].rearrange("p (t e) -> p t e", e=E))
            lm3 = lm[:].rearrange("p (t e) -> p t e", e=E)
            m2 = gat.tile([C, NT], F32)
            nc.vector.tensor_reduce(
                out=m2[:], in_=lm3, op=mybir.AluOpType.max, axis=mybir.AxisListType.X)
            ge2 = gat.tile([C, NT * E], F32)
            nc.vector.tensor_tensor(
                ge2[:].rearrange("p (t e) -> p t e", e=E), lm3,
                m2[:, :, None].broadcast_to((C, NT, E)),
                op=mybir.AluOpType.is_ge)
            dd = gat.tile([C, NT], F32)
            nc.vector.tensor_sub(dd[:], m2[:], m1[:])
            # sigmoid is not in the HW act-table set that has silu; use
            # sigmoid(z) = 0.5 + 0.5*tanh(z/2) (tanh shares silu's table).
            th = gat.tile([C, NT], F32)
            nc.scalar.activation(
                th[:], dd[:], mybir.ActivationFunctionType.Tanh, scale=-0.5)
            p1 = gat.tile([C, NT], F32)  # sigmoid(m1-m2)
            nc.vector.tensor_scalar(
                out=p1[:], in0=th[:], scalar1=0.5, scalar2=0.5,
                op0=mybir.AluOpType.mult, op1=mybir.AluOpType.add)
            p2 = gat.tile([C, NT], F32)  # sigmoid(m2-m1)
            nc.vector.tensor_scalar(
                out=p2[:], in0=th[:], scalar1=-0.5, scalar2=0.5,
                op0=mybir.AluOpType.mult, op1=mybir.AluOpType.add)
            # wt = ge1*p1 + ge2*p2  (voxel-major top-2 softmax weights)
            nc.vector.tensor_mul(
                g13, g13, p1[:, :, None].broadcast_to((C, NT, E)))
            nc.vector.tensor_mul(
                ge2[:].rearrange("p (t e) -> p t e", e=E),
                ge2[:].rearrange("p (t e) -> p t e", e=E),
                p2[:, :, None].broadcast_to((C, NT, E)))
            wt = gat.tile([C, NT * E], F32)
            wt_ins = nc.vector.tensor_add(wt[:], ge1[:], ge2[:])

            # transpose wt [128,(t,8)] -> channel-major [8, NSH] via PE,
            # bounce through SBUF (PSUM cannot be DMA'd) to DRAM
            wcm_sb = consts.tile([E, NSH], mybir.dt.bfloat16)
            TJ = 16  # psum transposes batched 4 per bank
            prev_copy = None
            prev_grp = []
            for j in range(TJ):
                pst = ps_l.tile([E, 4 * C], F32, tag="pst")
                grp = []
                for k in range(4):
                    ti = 4 * j + k
                    grp.append(nc.tensor.transpose(
                        pst[:, k * C:(k + 1) * C],
                        wt[:, ti * E:(ti + 1) * E],
                        ident[:]))
                pe_absorb([wt_ins if j == 0 else None, prev_copy,
                           prev_grp[-1] if prev_grp else None], grp[:1])
                for m in grp[1:]:
                    add_dep_helper(m.ins, grp[0].ins, sync=False)
                prev_copy = nc.scalar.copy(
                    wcm_sb[:, j * 4 * C:(j + 1) * 4 * C], pst[:])
                prev_grp = grp
            gpsum.__exit__(None, None, None)

            # ---------- phase M: experts + combine ----------
            mpsum = tc.tile_pool(name="ps_m", bufs=1, space="PSUM")
            ps_m = mpsum.__enter__()
            mpsum2 = tc.tile_pool(name="ps_m2", bufs=2, space="PSUM")
            ps_m2 = mpsum2.__enter__()
            prev_resid = prev_l2last = None
            hist_silu = [None, None]
            hist_mult = [None, None]
            hist_hmm = [None, None]
            hist_wb = [None, None]
            for i in range(NCHUNKS):
                cs = slice(i * NC_CHUNK, (i + 1) * NC_CHUNK)
                pso = ps_m.tile([C, NC_CHUNK], F32, tag="pso")
                for e in range(E):
                    mms = []
                    psh = ps_m2.tile([C, NC_CHUNK], F32, tag="psh")
                    for s in range(NC_CHUNK // 512):
                        rs = slice(i * NC_CHUNK + s * 512,
                                   i * NC_CHUNK + (s + 1) * 512)
                        mms.append(nc.tensor.matmul(
                            psh[:, s * 512:(s + 1) * 512],
                            w1T[:, e * C:(e + 1) * C],
                            x_sb[:, rs],
                            start=True, stop=True))
                    f_sb = fpool.tile([C, NC_CHUNK], F32, tag="f")
                    if hasb1:
                        silu_ins = nc.scalar.activation(
                            f_sb[:], psh[:], act_fn, bias=b1m[:, e:e + 1])
                    else:
                        silu_ins = nc.scalar.activation(f_sb[:], psh[:], act_fn)
                    pswb = ps_m.tile([C, NC_CHUNK], F32, tag="pswb")
                    for s in range(NC_CHUNK // 512):
                        ws = slice(i * NC_CHUNK + s * 512,
                                   i * NC_CHUNK + (s + 1) * 512)
                        mms.append(nc.tensor.matmul(
                            pswb[:, s * 512:(s + 1) * 512],
                            sel[:, e * C:(e + 1) * C], wcm_sb[:, ws],
                            start=True, stop=True))
                    g_sb = gpool.tile([C, NC_CHUNK], mybir.dt.bfloat16, tag="g")
                    mult_ins = nc.vector.tensor_mul(g_sb[:], f_sb[:], pswb[:])
                    for s in range(NC_CHUNK // 512):
                        ss = slice(s * 512, (s + 1) * 512)
                        mms.append(nc.tensor.matmul(
                            pso[:, ss],
                            w2T[:, e * C:(e + 1) * C],
                            g_sb[:, ss],
                            start=(e == 0),
                            stop=(e == E - 1) and not hasb2))
                    # absorb all cross-engine + psum-WAW deps into PE nops
                    pe_absorb([hist_silu[0], hist_mult[-1], prev_resid,
                               hist_hmm[0], hist_wb[-1], prev_l2last],
                              mms[:1])
                    for m in mms[1:]:
                        add_dep_helper(m.ins, mms[0].ins, sync=False)
                    pe_absorb([silu_ins, mult_ins], mms[-NC_CHUNK // 512:])
                    hist_silu = [hist_silu[-1], silu_ins]
                    hist_mult = [hist_mult[-1], mult_ins]
                    hist_hmm = [hist_hmm[-1], mms[1]]
                    hist_wb = [hist_wb[-1], mms[NC_CHUNK // 512 + 1]]
                    if e == E - 1:
                        prev_l2last = mms[-1]
                if hasb2:
                    for s in range(NC_CHUNK // 512):
                        ss = slice(s * 512, (s + 1) * 512)
                        rs = slice(i * NC_CHUNK + s * 512,
                                   i * NC_CHUNK + (s + 1) * 512)
                        nc.tensor.matmul(
                            pso[:, ss], b2m[:], wcm_sb[:, rs],
                            start=False, stop=True)
                o_sb = opool.tile([C, NC_CHUNK], F32, tag="o")
                prev_resid = nc.vector.tensor_add(
                    o_sb[:], pso[:], x_sb[:, cs].bitcast(F32))
                nc.sync.dma_start(out_d[:, cs], o_sb[:])
            mpsum2.__exit__(None, None, None)
            mpsum.__exit__(None, None, None)
    _split_waits(nc)
    return nc


_cache = {}


def _get_nc(key):
    if key not in _cache:
        _cache[key] = build_kernel(*key)
    return _cache[key]


def kernel(x, gate_w, gate_b, w1, b1, w2, b2, _trace=False):
    x = np.asarray(x, dtype=np.float32)
    gate_w = np.asarray(gate_w, dtype=np.float32)
    gate_b = np.asarray(gate_b, dtype=np.float32)
    w1 = np.asarray(w1, dtype=np.float32)
    b1 = np.asarray(b1, dtype=np.float32)
    w2 = np.asarray(w2, dtype=np.float32)
    b2 = np.asarray(b2, dtype=np.float32)

    x_cm = np.ascontiguousarray(
        x.transpose(1, 0, 2, 3, 4).reshape(C, NVOX))
    gwT = np.ascontiguousarray(gate_w.T)                      # [C, E]
    gbr = np.tile(gate_b[None, :], (C, 1)).astype(np.float32)  # [C, E]
    w1T = np.ascontiguousarray(w1.T)                          # [C, E*C]
    b1m = np.ascontiguousarray(b1.reshape(E, C).T)            # [C, E]
    w2T = np.ascontiguousarray(
        w2.transpose(2, 0, 1).reshape(C, E * C)).astype(ml_dtypes.bfloat16)
    b2m = np.ascontiguousarray(b2).astype(ml_dtypes.bfloat16)
    ident = np.eye(C, dtype=np.float32)
    sel = np.zeros((E, E * C), dtype=ml_dtypes.bfloat16)
    for e in range(E):
        sel[e, e * C:(e + 1) * C] = 1.0

    key = (bool(gate_b.any()), bool(b1.any()), bool(b2.any()))
    nc = _get_nc(key)

    in_maps = []
    for c in range(NCORES):
        sh = np.ascontiguousarray(x_cm[:, c * NSH:(c + 1) * NSH])
        in_maps.append({
            "x": sh, "gwT": gwT, "gb": gbr, "w1T": w1T, "b1m": b1m,
            "w2T": w2T, "b2m": b2m, "ident": ident, "sel": sel,
        })

    res = run_bass_kernel_spmd(
        nc, in_maps, core_ids=list(range(NCORES)), trace=_trace)
    out_cm = np.concatenate([res.results[c]["out"] for c in range(NCORES)],
                            axis=1)
    out = out_cm.reshape(C, B, T, H, W).transpose(1, 0, 2, 3, 4)
    out = np.ascontiguousarray(out, dtype=np.float32)
    if _trace:
        return out, res
    return out
